# revision 14
# baseline (speedup 1.0000x reference)
"""BertAttention (B=4, S=2048, HID=1024, NH=16) on 8 TRN2 NeuronCores.

Sharding: core = (batch b, head-group g); b = core//2, g = core%2.
Each core handles batch b, heads [g*8, (g+1)*8) == channels [g*512, (g+1)*512).

Per-core dataflow (bf16 matmul operands, f32 PSUM accumulation, f32 outputs):
  phase 0: load W slices + activations, cast bf16, PE-transpose to put the
           contraction dim on partitions; project to qT/kT [c, i] bf16
           (q pre-scaled by 1/8 via host-side weight scaling) and
           v_aug [j, (h, 65)] bf16 with a ones column per head (softmax
           denominator rides the ctx matmul).
  per i-block of 1024 rows:
    mask rows loaded once (int32 -> bf16), kept for both parts.
    B-part: scores^T = kT_h.T @ qT_h in [j, i] layout, exp on ScalarE
            (no max-subtraction: |s| = O(10)), multiplied by the transposed
            0/1 mask, then ctx^T_aug = v_aug.T @ EM accumulated over j in
            PSUM; row 64 is the softmax denominator. Finalize: PE-transpose
            ctx^T to natural layout, multiply by 1/denominator per partition.
    A-part: scores_h = qT_h.T @ kT_h in natural [i, j] layout, masked
            additively with A = (m-1)*1e9 (exact f32 0/-1e9) on VectorE,
            streamed to the scores output.
"""
import functools
import numpy as np

import concourse.bass as bass
import concourse.mybir as mybir
import concourse.tile as tile
import concourse.bacc as bacc
from concourse import bass_utils
from concourse.masks import make_identity

FP32 = mybir.dt.float32
BF16 = mybir.dt.bfloat16
I32 = mybir.dt.int32
AF = mybir.ActivationFunctionType

S = 2048
F = 1024          # hidden (contraction for projections)
C = 512           # channels per core (8 heads x 64)
NHC = 8           # heads per core
HD = 64
NEG = -1.0e9


def _emit_projection(nc, wp, pps, xd, wd, bd, ident_b, kind, xT, WT, b_sb, ones_b, b_row_b, qT=None, v_aug=None):
    """Load W+bias+x for one tensor, cast bf16, transpose, project.
    kind: 'qk' -> writes qT [128, 4, S] bf16 (bias via ACT Identity);
          'v'  -> writes v_aug [128, 16, 8, 65] bf16 (bias via K=1 matmul).
    """
    # --- W natural -> WT [128f, 8fc, C] bf16
    for cc in range(4):
        wnat = wp.tile([128, F], FP32, tag="wnat")
        nc.sync.dma_start(out=wnat[:], in_=wd[cc * 128:(cc + 1) * 128, :])
        wnb = wp.tile([128, F], BF16, tag="wnb")
        nc.scalar.copy(wnb[:], wnat[:])
        for fq in range(2):
            ps = pps.tile([128, 512], BF16, tag="p0ps")
            for f2 in range(4):
                nc.tensor.transpose(ps[:, f2 * 128:(f2 + 1) * 128],
                                    wnb[:, (fq * 4 + f2) * 128:(fq * 4 + f2 + 1) * 128],
                                    ident_b[:])
            nc.vector.tensor_copy(WT[:, fq * 4:(fq + 1) * 4, cc * 128:(cc + 1) * 128],
                                  ps[:].rearrange("p (a b) -> p a b", a=4))
    # --- bias
    if kind == "qk":
        nc.sync.dma_start(out=b_sb[:], in_=bd.rearrange("(c p) -> p c", p=128))
    else:
        b_row = wp.tile([1, C], FP32, tag="brow")
        nc.sync.dma_start(out=b_row[:], in_=bd.rearrange("(a c) -> a c", a=1))
        nc.vector.tensor_copy(b_row_b[:], b_row[:])
    # --- x natural -> xT [128f, 8fc, S] bf16
    for it in range(16):
        xnat = wp.tile([128, F], FP32, tag="xnat")
        nc.sync.dma_start(out=xnat[:], in_=xd[it * 128:(it + 1) * 128, :])
        xnb = wp.tile([128, F], BF16, tag="xnb")
        nc.scalar.copy(xnb[:], xnat[:])
        for fq in range(2):
            ps = pps.tile([128, 512], BF16, tag="p0ps")
            for f2 in range(4):
                nc.tensor.transpose(ps[:, f2 * 128:(f2 + 1) * 128],
                                    xnb[:, (fq * 4 + f2) * 128:(fq * 4 + f2 + 1) * 128],
                                    ident_b[:])
            nc.vector.tensor_copy(xT[:, fq * 4:(fq + 1) * 4, it * 128:(it + 1) * 128],
                                  ps[:].rearrange("p (a b) -> p a b", a=4))
    # --- project
    if kind == "qk":
        for cc in range(4):
            for ibp in range(4):
                ps = pps.tile([128, 512], FP32, tag="p0mm")
                for k in range(8):
                    nc.tensor.matmul(ps[:], WT[:, k, cc * 128:(cc + 1) * 128],
                                     xT[:, k, ibp * 512:(ibp + 1) * 512],
                                     start=(k == 0), stop=(k == 7))
                nc.scalar.activation(qT[:, cc, ibp * 512:(ibp + 1) * 512], ps[:],
                                     AF.Identity, bias=b_sb[:, cc:cc + 1], scale=1.0)
    else:
        for jt in range(16):
            ps = pps.tile([128, 512], FP32, tag="p0mm")
            nc.tensor.matmul(ps[:], ones_b[:, 0:128], b_row_b[:], start=True, stop=False)
            for k in range(8):
                nc.tensor.matmul(ps[:], xT[:, k, jt * 128:(jt + 1) * 128], WT[:, k, :],
                                 start=False, stop=(k == 7))
            nc.scalar.copy(v_aug[:, jt, :, 0:64],
                           ps[:].rearrange("p (h d) -> p h d", h=8))


def build_attn():
    nc = bacc.Bacc("TRN2", target_bir_lowering=False)
    xq_d = nc.dram_tensor("xq", [S, F], FP32, kind="ExternalInput")
    xk_d = nc.dram_tensor("xk", [S, F], FP32, kind="ExternalInput")
    xv_d = nc.dram_tensor("xv", [S, F], FP32, kind="ExternalInput")
    m_d = nc.dram_tensor("mask", [S, S], I32, kind="ExternalInput")
    wq_d = nc.dram_tensor("wq", [C, F], FP32, kind="ExternalInput")
    wk_d = nc.dram_tensor("wk", [C, F], FP32, kind="ExternalInput")
    wv_d = nc.dram_tensor("wv", [C, F], FP32, kind="ExternalInput")
    bq_d = nc.dram_tensor("bq", [C], FP32, kind="ExternalInput")
    bk_d = nc.dram_tensor("bk", [C], FP32, kind="ExternalInput")
    bv_d = nc.dram_tensor("bv", [C], FP32, kind="ExternalInput")
    sc_d = nc.dram_tensor("scores", [NHC, S, S], FP32, kind="ExternalOutput")
    cx_d = nc.dram_tensor("ctx", [S, C], FP32, kind="ExternalOutput")

    with tile.TileContext(nc) as tc:
        with tc.tile_pool(name="const", bufs=1) as cpool, \
             tc.tile_pool(name="persist", bufs=1) as pp:
            ident = cpool.tile([128, 128], FP32)
            make_identity(nc, ident[:])
            ident_b = cpool.tile([128, 128], BF16)
            nc.vector.tensor_copy(ident_b[:], ident[:])
            ones_f = cpool.tile([1, 128], FP32)
            nc.vector.memset(ones_f[:], 1.0)
            ones_b = cpool.tile([1, 128], BF16)
            nc.vector.tensor_copy(ones_b[:], ones_f[:])
            negb = cpool.tile([128, 1], FP32)
            nc.vector.memset(negb[:], NEG)

            qT = pp.tile([128, 4, S], BF16)
            kT = pp.tile([128, 4, S], BF16)
            v_aug = pp.tile([128, 16, NHC, 65], BF16)
            nc.vector.memset(v_aug[:, :, :, 64:65], 1.0)

            # ---------------- phase 0: projections ----------------
            with tc.tile_pool(name="p0w", bufs=3) as wp, \
                 tc.tile_pool(name="p0pp", bufs=1) as p0pp, \
                 tc.tile_pool(name="p0ps", bufs=4, space="PSUM") as pps:
                xT = p0pp.tile([128, 8, S], BF16)
                WT = p0pp.tile([128, 8, C], BF16)
                b_sb = p0pp.tile([128, 4], FP32)
                b_row_b = p0pp.tile([1, C], BF16)
                _emit_projection(nc, wp, pps, xq_d, wq_d, bq_d, ident_b, "qk",
                                 xT, WT, b_sb, ones_b, b_row_b, qT=qT)
                _emit_projection(nc, wp, pps, xk_d, wk_d, bk_d, ident_b, "qk",
                                 xT, WT, b_sb, ones_b, b_row_b, qT=kT)
                _emit_projection(nc, wp, pps, xv_d, wv_d, bv_d, ident_b, "v",
                                 xT, WT, b_sb, ones_b, b_row_b, v_aug=v_aug)

            # ---------------- attention ----------------
            with tc.tile_pool(name="aw", bufs=2) as aw, \
                 tc.tile_pool(name="mtp", bufs=1) as mtp, \
                 tc.tile_pool(name="em", bufs=2) as emp, \
                 tc.tile_pool(name="stg", bufs=2) as stg, \
                 tc.tile_pool(name="fin", bufs=2) as fin, \
                 tc.tile_pool(name="psSA", bufs=1, space="PSUM") as psa, \
                 tc.tile_pool(name="psST", bufs=1, space="PSUM") as pst, \
                 tc.tile_pool(name="psCX", bufs=1, space="PSUM") as pcx:
                for ib in range(2):   # i-blocks of 1024
                    # ---- load mask rows for this i-block once (int32 -> bf16)
                    mb = mtp.tile([128, 8, S], BF16, tag="mbib")
                    for ii in range(8):
                        ml = aw.tile([128, S], I32, tag="mload")
                        nc.sync.dma_start(
                            out=ml[:], in_=m_d[(ib * 8 + ii) * 128:(ib * 8 + ii + 1) * 128, :])
                        nc.scalar.copy(mb[:, ii, :], ml[:])
                    # ---- build mT for this i-block ----
                    mT = mtp.tile([128, 16, 1024], BF16, tag="mT")
                    for ii in range(8):
                        for jq in range(4):
                            ps = pst.tile([128, 512], BF16, tag="stp")
                            for f2 in range(4):
                                nc.tensor.transpose(ps[:, f2 * 128:(f2 + 1) * 128],
                                                    mb[:, ii, (jq * 4 + f2) * 128:(jq * 4 + f2 + 1) * 128],
                                                    ident_b[:])
                            nc.vector.tensor_copy(
                                mT[:, jq * 4:(jq + 1) * 4, ii * 128:(ii + 1) * 128],
                                ps[:].rearrange("p (a b) -> p a b", a=4))
                    # ---- per-head B-part with interleaved A-part steps ----
                    for h in range(NHC):
                        hp, hi = h // 2, h % 2
                        pb = hi * 64   # partition base of this head in qT/kT chunks
                        it = ib * 8 + h      # A-part i-tile paired with this head
                        arow = aw.tile([128, S], FP32, tag="arow")
                        nc.scalar.activation(arow[:], mb[:, h, :], AF.Identity,
                                             bias=negb[:], scale=1.0e9)
                        cxp = pcx.tile([65, 1024], FP32, tag="ctxp")
                        stgs = None
                        for jt in range(16):
                            # ---- B-step ----
                            st = pst.tile([128, 1024], FP32, tag="stp")
                            for nh_ in range(2):
                                nc.tensor.matmul(
                                    st[:, nh_ * 512:(nh_ + 1) * 512],
                                    kT[pb:pb + 64, hp, jt * 128:(jt + 1) * 128],
                                    qT[pb:pb + 64, hp, ib * 1024 + nh_ * 512: ib * 1024 + (nh_ + 1) * 512],
                                    start=True, stop=True, tile_position=(pb, 0))
                            e = emp.tile([128, 1024], BF16, tag="e")
                            nc.scalar.activation(e[:], st[:], AF.Exp)
                            em = emp.tile([128, 1024], BF16, tag="em")
                            nc.vector.tensor_mul(em[:], e[:], mT[:, jt, :])
                            for nh_ in range(2):
                                nc.tensor.matmul(
                                    cxp[:, nh_ * 512:(nh_ + 1) * 512],
                                    v_aug[:, jt, h, :],
                                    em[:, nh_ * 512:(nh_ + 1) * 512],
                                    start=(jt == 0), stop=(jt == 15))
                            # ---- A-step every other jt: 8 steps = 4 hpA x 2 jh ----
                            if jt % 2 == 1:
                                astep = jt // 2
                                hpA, jh = astep // 2, astep % 2
                                if jh == 0:
                                    stg_a = stg.tile([128, S], FP32, tag="stg0")
                                    stg_b = stg.tile([128, S], FP32, tag="stg1")
                                    stgs = [stg_a, stg_b]
                                for hiA in range(2):
                                    pbA = hiA * 64
                                    sp = psa.tile([128, 1024], FP32, tag=f"sps{hiA}")
                                    for nh_ in range(2):
                                        nc.tensor.matmul(
                                            sp[:, nh_ * 512:(nh_ + 1) * 512],
                                            qT[pbA:pbA + 64, hpA, it * 128:(it + 1) * 128],
                                            kT[pbA:pbA + 64, hpA, jh * 1024 + nh_ * 512: jh * 1024 + (nh_ + 1) * 512],
                                            start=True, stop=True, tile_position=(pbA, 0))
                                    nc.vector.tensor_add(
                                        stgs[hiA][:, jh * 1024:(jh + 1) * 1024],
                                        sp[:], arow[:, jh * 1024:(jh + 1) * 1024])
                                if jh == 1:
                                    for hiA in range(2):
                                        nc.sync.dma_start(
                                            out=sc_d[2 * hpA + hiA, it * 128:(it + 1) * 128, :],
                                            in_=stgs[hiA][:])
                        # ---- finalize head: transpose + normalize + out (bf16) ----
                        cu = fin.tile([65, 1024], BF16, tag="cu")
                        nc.scalar.copy(cu[:], cxp[:])
                        for blk in range(8):
                            tp = pst.tile([128, 65], BF16, tag="stp")
                            nc.tensor.transpose(tp[:], cu[:, blk * 128:(blk + 1) * 128],
                                                ident_b[0:65, 0:65])
                            rc = fin.tile([128, 1], FP32, tag="rc")
                            nc.vector.reciprocal(rc[:], tp[:, 64:65])
                            ot = fin.tile([128, 64], FP32, tag="ot")
                            nc.vector.tensor_scalar_mul(ot[:], tp[:, 0:64], rc[:])
                            r0 = ib * 1024 + blk * 128
                            nc.sync.dma_start(out=cx_d[r0:r0 + 128, h * 64:(h + 1) * 64],
                                              in_=ot[:])
    nc.compile()
    return nc


@functools.lru_cache(maxsize=1)
def _get_nc():
    return build_attn()


def make_in_maps(inputs):
    return _make_in_maps(**inputs)


def _make_in_maps(query, key, value, attention_mask, Wq, bq, Wk, bk, Wv, bv):
    query = np.asarray(query, dtype=np.float32)
    key = np.asarray(key, dtype=np.float32)
    value = np.asarray(value, dtype=np.float32)
    attention_mask = np.asarray(attention_mask, dtype=np.int32)
    in_maps = []
    for core in range(8):
        b, g = core // 2, core % 2
        sl = slice(g * C, (g + 1) * C)
        in_maps.append({
            "xq": np.ascontiguousarray(query[b]),
            "xk": np.ascontiguousarray(key[b]),
            "xv": np.ascontiguousarray(value[b]),
            "mask": np.ascontiguousarray(attention_mask[b]),
            "wq": np.ascontiguousarray(np.asarray(Wq, np.float32)[sl] / 8.0),
            "wk": np.ascontiguousarray(np.asarray(Wk, np.float32)[sl]),
            "wv": np.ascontiguousarray(np.asarray(Wv, np.float32)[sl]),
            "bq": np.ascontiguousarray(np.asarray(bq, np.float32)[sl] / 8.0),
            "bk": np.ascontiguousarray(np.asarray(bk, np.float32)[sl]),
            "bv": np.ascontiguousarray(np.asarray(bv, np.float32)[sl]),
        })
    return in_maps


def kernel(query, key, value, attention_mask, Wq, bq, Wk, bk, Wv, bv):
    nc = _get_nc()
    in_maps = _make_in_maps(query, key, value, attention_mask, Wq, bq, Wk, bk, Wv, bv)
    res = bass_utils.run_bass_kernel_spmd(nc, in_maps, core_ids=list(range(8)))
    scores = np.empty((4, 16, S, S), np.float32)
    ctx = np.empty((4, S, F), np.float32)
    for core in range(8):
        b, g = core // 2, core % 2
        scores[b, g * NHC:(g + 1) * NHC] = res.results[core]["scores"]
        ctx[b, :, g * C:(g + 1) * C] = res.results[core]["ctx"]
    return ctx, scores


# revision 19
# speedup vs baseline: 1.2735x; 1.2735x over previous
"""BertAttention (B=4, S=2048, HID=1024, NH=16) on 8 TRN2 NeuronCores.

Sharding: core = (batch b, head-group g); b = core//2, g = core%2.
Each core handles batch b, heads [g*8, (g+1)*8) == channels [g*512, (g+1)*512).

Per-core dataflow (bf16 matmul operands, f32 PSUM accumulation, f32 outputs):
  phase 0: load W slices + activations, cast bf16, PE-transpose to put the
           contraction dim on partitions; project to qT/kT [c, i] bf16
           (q pre-scaled by 1/8 via host-side weight scaling) and
           v_aug [j, (h, 65)] bf16 with a ones column per head (softmax
           denominator rides the ctx matmul).
  per i-block of 1024 rows:
    mask rows loaded once (int32 -> bf16), kept for both parts.
    B-part: scores^T = kT_h.T @ qT_h in [j, i] layout, exp on ScalarE
            (no max-subtraction: |s| = O(10)), multiplied by the transposed
            0/1 mask, then ctx^T_aug = v_aug.T @ EM accumulated over j in
            PSUM; row 64 is the softmax denominator. Finalize: PE-transpose
            ctx^T to natural layout, multiply by 1/denominator per partition.
    A-part: scores_h = qT_h.T @ kT_h in natural [i, j] layout, masked
            additively with A = (m-1)*1e9 (exact f32 0/-1e9) on VectorE,
            streamed to the scores output.
"""
import functools
import numpy as np

import concourse.bass as bass
import concourse.mybir as mybir
import concourse.tile as tile
import concourse.bacc as bacc
from concourse import bass_utils
from concourse.masks import make_identity

FP32 = mybir.dt.float32
BF16 = mybir.dt.bfloat16
I32 = mybir.dt.int32
AF = mybir.ActivationFunctionType

S = 2048
F = 1024          # hidden (contraction for projections)
C = 512           # channels per core (8 heads x 64)
NHC = 8           # heads per core
HD = 64
NEG = -1.0e9


def _emit_projection(nc, wp, pps, xd, wd, bd, ident_b, kind, xT, WT, b_sb, ones_b, b_row_b, qT=None, v_aug=None):
    """Load W+bias+x for one tensor, cast bf16, transpose, project.
    kind: 'qk' -> writes qT [128, 4, S] bf16 (bias via ACT Identity);
          'v'  -> writes v_aug [128, 16, 8, 65] bf16 (bias via K=1 matmul).
    """
    # --- W natural -> WT [128f, 8fc, C] bf16
    for cc in range(4):
        wnat = wp.tile([128, F], FP32, tag="wnat")
        nc.sync.dma_start(out=wnat[:], in_=wd[cc * 128:(cc + 1) * 128, :])
        wnb = wp.tile([128, F], BF16, tag="wnb")
        nc.scalar.copy(wnb[:], wnat[:])
        for fq in range(2):
            ps = pps.tile([128, 512], BF16, tag="p0ps")
            for f2 in range(4):
                nc.tensor.transpose(ps[:, f2 * 128:(f2 + 1) * 128],
                                    wnb[:, (fq * 4 + f2) * 128:(fq * 4 + f2 + 1) * 128],
                                    ident_b[:])
            nc.vector.tensor_copy(WT[:, fq * 4:(fq + 1) * 4, cc * 128:(cc + 1) * 128],
                                  ps[:].rearrange("p (a b) -> p a b", a=4))
    # --- bias
    if kind == "qk":
        nc.sync.dma_start(out=b_sb[:], in_=bd.rearrange("(c p) -> p c", p=128))
    else:
        b_row = wp.tile([1, C], FP32, tag="brow")
        nc.sync.dma_start(out=b_row[:], in_=bd.rearrange("(a c) -> a c", a=1))
        nc.vector.tensor_copy(b_row_b[:], b_row[:])
    # --- x natural -> xT [128f, 8fc, S] bf16
    for it in range(16):
        xnat = wp.tile([128, F], FP32, tag="xnat")
        nc.sync.dma_start(out=xnat[:], in_=xd[it * 128:(it + 1) * 128, :])
        xnb = wp.tile([128, F], BF16, tag="xnb")
        nc.scalar.copy(xnb[:], xnat[:])
        for fq in range(2):
            ps = pps.tile([128, 512], BF16, tag="p0ps")
            for f2 in range(4):
                nc.tensor.transpose(ps[:, f2 * 128:(f2 + 1) * 128],
                                    xnb[:, (fq * 4 + f2) * 128:(fq * 4 + f2 + 1) * 128],
                                    ident_b[:])
            nc.vector.tensor_copy(xT[:, fq * 4:(fq + 1) * 4, it * 128:(it + 1) * 128],
                                  ps[:].rearrange("p (a b) -> p a b", a=4))
    # --- project
    if kind == "qk":
        for cc in range(4):
            for ibp in range(4):
                ps = pps.tile([128, 512], FP32, tag="p0mm")
                for k in range(8):
                    nc.tensor.matmul(ps[:], WT[:, k, cc * 128:(cc + 1) * 128],
                                     xT[:, k, ibp * 512:(ibp + 1) * 512],
                                     start=(k == 0), stop=(k == 7))
                nc.scalar.activation(qT[:, cc, ibp * 512:(ibp + 1) * 512], ps[:],
                                     AF.Identity, bias=b_sb[:, cc:cc + 1], scale=1.0)
    else:
        for jt in range(16):
            ps = pps.tile([128, 512], FP32, tag="p0mm")
            nc.tensor.matmul(ps[:], ones_b[:, 0:128], b_row_b[:], start=True, stop=False)
            for k in range(8):
                nc.tensor.matmul(ps[:], xT[:, k, jt * 128:(jt + 1) * 128], WT[:, k, :],
                                 start=False, stop=(k == 7))
            nc.scalar.copy(v_aug[:, jt, :, 0:64],
                           ps[:].rearrange("p (h d) -> p h d", h=8))


def build_attn():
    nc = bacc.Bacc("TRN2", target_bir_lowering=False)
    xq_d = nc.dram_tensor("xq", [S, F], FP32, kind="ExternalInput")
    xk_d = nc.dram_tensor("xk", [S, F], FP32, kind="ExternalInput")
    xv_d = nc.dram_tensor("xv", [S, F], FP32, kind="ExternalInput")
    m_d = nc.dram_tensor("mask", [S, S], I32, kind="ExternalInput")
    wq_d = nc.dram_tensor("wq", [C, F], FP32, kind="ExternalInput")
    wk_d = nc.dram_tensor("wk", [C, F], FP32, kind="ExternalInput")
    wv_d = nc.dram_tensor("wv", [C, F], FP32, kind="ExternalInput")
    bq_d = nc.dram_tensor("bq", [C], FP32, kind="ExternalInput")
    bk_d = nc.dram_tensor("bk", [C], FP32, kind="ExternalInput")
    bv_d = nc.dram_tensor("bv", [C], FP32, kind="ExternalInput")
    sc_d = nc.dram_tensor("scores", [NHC, S, S], FP32, kind="ExternalOutput")
    cx_d = nc.dram_tensor("ctx", [S, C], FP32, kind="ExternalOutput")

    with tile.TileContext(nc) as tc:
        with tc.tile_pool(name="const", bufs=1) as cpool, \
             tc.tile_pool(name="persist", bufs=1) as pp:
            ident = cpool.tile([128, 128], FP32)
            make_identity(nc, ident[:])
            ident_b = cpool.tile([128, 128], BF16)
            nc.vector.tensor_copy(ident_b[:], ident[:])
            ones_f = cpool.tile([1, 128], FP32)
            nc.vector.memset(ones_f[:], 1.0)
            ones_b = cpool.tile([1, 128], BF16)
            nc.vector.tensor_copy(ones_b[:], ones_f[:])
            negb = cpool.tile([128, 1], FP32)
            nc.vector.memset(negb[:], NEG)

            qT = pp.tile([128, 4, S], BF16)
            kT = pp.tile([128, 4, S], BF16)
            v_aug = pp.tile([128, 16, NHC, 65], BF16)
            nc.vector.memset(v_aug[:, :, :, 64:65], 1.0)

            # ---------------- phase 0: projections ----------------
            with tc.tile_pool(name="p0w", bufs=3) as wp, \
                 tc.tile_pool(name="p0pp", bufs=1) as p0pp, \
                 tc.tile_pool(name="p0ps", bufs=4, space="PSUM") as pps:
                xT = p0pp.tile([128, 8, S], BF16)
                WT = p0pp.tile([128, 8, C], BF16)
                b_sb = p0pp.tile([128, 4], FP32)
                b_row_b = p0pp.tile([1, C], BF16)
                _emit_projection(nc, wp, pps, xq_d, wq_d, bq_d, ident_b, "qk",
                                 xT, WT, b_sb, ones_b, b_row_b, qT=qT)
                _emit_projection(nc, wp, pps, xk_d, wk_d, bk_d, ident_b, "qk",
                                 xT, WT, b_sb, ones_b, b_row_b, qT=kT)
                _emit_projection(nc, wp, pps, xv_d, wv_d, bv_d, ident_b, "v",
                                 xT, WT, b_sb, ones_b, b_row_b, v_aug=v_aug)

            # ---------------- attention ----------------
            with tc.tile_pool(name="aw", bufs=2) as aw, \
                 tc.tile_pool(name="mtp", bufs=1) as mtp, \
                 tc.tile_pool(name="em", bufs=2) as emp, \
                 tc.tile_pool(name="stg", bufs=2) as stg, \
                 tc.tile_pool(name="fin", bufs=2) as fin, \
                 tc.tile_pool(name="psSA", bufs=1, space="PSUM") as psa, \
                 tc.tile_pool(name="psST", bufs=2, space="PSUM") as pst, \
                 tc.tile_pool(name="psCX", bufs=1, space="PSUM") as pcx:
                for ib in range(2):   # i-blocks of 1024
                    # ---- load mask rows for this i-block once (int32 -> bf16)
                    mb = mtp.tile([128, 8, S], BF16, tag="mbib")
                    for ii in range(8):
                        ml = aw.tile([128, S], I32, tag="mload")
                        nc.sync.dma_start(
                            out=ml[:], in_=m_d[(ib * 8 + ii) * 128:(ib * 8 + ii + 1) * 128, :])
                        nc.scalar.copy(mb[:, ii, :], ml[:])
                    # ---- build mT for this i-block ----
                    mT = mtp.tile([128, 16, 1024], BF16, tag="mT")
                    for ii in range(8):
                        for jq in range(4):
                            ps = pst.tile([128, 512], BF16, tag="stp")
                            for f2 in range(4):
                                nc.tensor.transpose(ps[:, f2 * 128:(f2 + 1) * 128],
                                                    mb[:, ii, (jq * 4 + f2) * 128:(jq * 4 + f2 + 1) * 128],
                                                    ident_b[:])
                            nc.vector.tensor_copy(
                                mT[:, jq * 4:(jq + 1) * 4, ii * 128:(ii + 1) * 128],
                                ps[:].rearrange("p (a b) -> p a b", a=4))
                    # ---- per-head B-part with one interleaved A-drain per jt ----
                    for h in range(NHC):
                        hp, hi = h // 2, h % 2
                        pb = hi * 64   # partition base of this head in qT/kT chunks
                        it = ib * 8 + h      # A-part i-tile paired with this head
                        arow = aw.tile([128, S], FP32, tag="arow")
                        nc.scalar.activation(arow[:], mb[:, h, :], AF.Identity,
                                             bias=negb[:], scale=1.0e9)
                        cxp = pcx.tile([65, 1024], FP32, tag="ctxp")
                        stg_c = None
                        for jt in range(16):
                            # ---- B-step ----
                            st = pst.tile([128, 1024], FP32, tag="stp")
                            for nh_ in range(2):
                                nc.tensor.matmul(
                                    st[:, nh_ * 512:(nh_ + 1) * 512],
                                    kT[pb:pb + 64, hp, jt * 128:(jt + 1) * 128],
                                    qT[pb:pb + 64, hp, ib * 1024 + nh_ * 512: ib * 1024 + (nh_ + 1) * 512],
                                    start=True, stop=True, tile_position=(pb, 0))
                            e = emp.tile([128, 1024], BF16, tag="e")
                            nc.scalar.activation(e[:], st[:], AF.Exp)
                            em = emp.tile([128, 1024], BF16, tag="em")
                            nc.vector.tensor_mul(em[:], e[:], mT[:, jt, :])
                            for nh_ in range(2):
                                nc.tensor.matmul(
                                    cxp[:, nh_ * 512:(nh_ + 1) * 512],
                                    v_aug[:, jt, h, :],
                                    em[:, nh_ * 512:(nh_ + 1) * 512],
                                    start=(jt == 0), stop=(jt == 15))
                            # ---- A-drain: (hpA, hiA, jh) = jt decomposition ----
                            jh = jt % 2
                            hiA = (jt % 4) // 2
                            hpA = jt // 4
                            pbA = hiA * 64
                            if jh == 0:
                                stg_c = stg.tile([128, S], FP32, tag="stg")
                            sp = psa.tile([128, 1024], FP32, tag="sps")
                            for nh_ in range(2):
                                nc.tensor.matmul(
                                    sp[:, nh_ * 512:(nh_ + 1) * 512],
                                    qT[pbA:pbA + 64, hpA, it * 128:(it + 1) * 128],
                                    kT[pbA:pbA + 64, hpA, jh * 1024 + nh_ * 512: jh * 1024 + (nh_ + 1) * 512],
                                    start=True, stop=True, tile_position=(pbA, 0))
                            nc.vector.tensor_add(
                                stg_c[:, jh * 1024:(jh + 1) * 1024],
                                sp[:], arow[:, jh * 1024:(jh + 1) * 1024])
                            if jh == 1:
                                nc.sync.dma_start(
                                    out=sc_d[2 * hpA + hiA, it * 128:(it + 1) * 128, :],
                                    in_=stg_c[:])
                        # ---- finalize head (batched): ctx rows bf16, denom f32 ----
                        cu = fin.tile([64, 1024], BF16, tag="cu")
                        nc.scalar.copy(cu[:], cxp[0:64, :])
                        du = fin.tile([1, 1024], FP32, tag="du")
                        nc.scalar.copy(du[:], cxp[64:65, :])
                        dnp = pst.tile([128, 8], FP32, tag="stp")
                        for blk in range(8):
                            nc.tensor.transpose(dnp[:, blk:blk + 1],
                                                du[0:1, blk * 128:(blk + 1) * 128],
                                                ident[0:1, 0:1])
                        dn = fin.tile([128, 8], FP32, tag="dn")
                        nc.vector.tensor_copy(dn[:], dnp[:])
                        tp = pst.tile([128, 8, 64], BF16, tag="stp")
                        for blk in range(8):
                            nc.tensor.transpose(tp[:, blk, :], cu[:, blk * 128:(blk + 1) * 128],
                                                ident_b[0:64, 0:64])
                        rc = fin.tile([128, 8], FP32, tag="rc")
                        nc.vector.reciprocal(rc[:], dn[:])
                        ot = fin.tile([128, 8, 64], FP32, tag="ot")
                        nc.vector.tensor_mul(ot[:], tp[:],
                                             rc[:].rearrange("p (a o) -> p a o", o=1).broadcast_to([128, 8, 64]))
                        nc.sync.dma_start(
                            out=cx_d[ib * 1024:(ib + 1) * 1024, h * 64:(h + 1) * 64].rearrange(
                                "(a p) d -> p a d", p=128),
                            in_=ot[:])
    nc.compile()
    return nc


@functools.lru_cache(maxsize=1)
def _get_nc():
    return build_attn()


def make_in_maps(inputs):
    return _make_in_maps(**inputs)


def _make_in_maps(query, key, value, attention_mask, Wq, bq, Wk, bk, Wv, bv):
    query = np.asarray(query, dtype=np.float32)
    key = np.asarray(key, dtype=np.float32)
    value = np.asarray(value, dtype=np.float32)
    attention_mask = np.asarray(attention_mask, dtype=np.int32)
    in_maps = []
    for core in range(8):
        b, g = core // 2, core % 2
        sl = slice(g * C, (g + 1) * C)
        in_maps.append({
            "xq": np.ascontiguousarray(query[b]),
            "xk": np.ascontiguousarray(key[b]),
            "xv": np.ascontiguousarray(value[b]),
            "mask": np.ascontiguousarray(attention_mask[b]),
            "wq": np.ascontiguousarray(np.asarray(Wq, np.float32)[sl] / 8.0),
            "wk": np.ascontiguousarray(np.asarray(Wk, np.float32)[sl]),
            "wv": np.ascontiguousarray(np.asarray(Wv, np.float32)[sl]),
            "bq": np.ascontiguousarray(np.asarray(bq, np.float32)[sl] / 8.0),
            "bk": np.ascontiguousarray(np.asarray(bk, np.float32)[sl]),
            "bv": np.ascontiguousarray(np.asarray(bv, np.float32)[sl]),
        })
    return in_maps


def kernel(query, key, value, attention_mask, Wq, bq, Wk, bk, Wv, bv):
    nc = _get_nc()
    in_maps = _make_in_maps(query, key, value, attention_mask, Wq, bq, Wk, bk, Wv, bv)
    res = bass_utils.run_bass_kernel_spmd(nc, in_maps, core_ids=list(range(8)))
    scores = np.empty((4, 16, S, S), np.float32)
    ctx = np.empty((4, S, F), np.float32)
    for core in range(8):
        b, g = core // 2, core % 2
        scores[b, g * NHC:(g + 1) * NHC] = res.results[core]["scores"]
        ctx[b, :, g * C:(g + 1) * C] = res.results[core]["ctx"]
    return ctx, scores
